# revision 1
# baseline (speedup 1.0000x reference)
"""LIF neuron scan kernel for Trainium2 (8 NeuronCores).

Problem: x[B=32, T=64, N=16384] f32, per-neuron thresh/tau_x[N].
    sig = sigmoid(tau_x)
    for t: mem = (x_t - mem)*sig + mem ; spike = (mem >= thresh) ; mem = (1-spike)*mem
Outputs: (spikes[B,T,N], mems[B,T,N]) both f32.

Sharding: data-parallel over batch B across 8 cores (4 batches/core);
thresh/tau_x replicated. The recurrence is only over T, elementwise over
(B, N), so cores are fully independent (no collectives).

Per-core layout: SBUF tiles [128, 512] per timestep with
  partition p = b_local*32 + sub   (b_local in [0,4), sub in [0,32))
  free      f = n_low in [0,512),  neuron n = sub*512 + n_low
so every DMA is a clean 2KB-contiguous-per-partition pattern.

Raw Bass (no Tile): this toolchain's walrus rejects instructions carrying
more than one attached sync command, so all synchronization is standalone
wait_ge instructions + one then_inc per instruction. DMA completion sems
are only ever waited at full-count values, with issue-side throttling so
increments from different transfers never race a partial-count target.

Engine split:
  SP   — input DMAs (x), one 512KB DMA per (block, local batch)
  ACT  — sigmoid(tau_x) once; output DMAs (spikes, mems)
  DVE  — the per-timestep 5-op chain:
           d  = x_t - mem        (tensor_sub)
           e  = d * sig          (tensor_mul)
           m' = e + mem          (tensor_add)
           s  = (m' >= thresh)   (tensor_tensor is_ge -> 1.0/0.0)
           m  = (s < 1) * m'     (scalar_tensor_tensor, hard reset)
         (t=0 runs the reduced chain e = x*sig since mem=0.)
Timesteps run in blocks of TBLK=8; x/spike/mem SBUF blocks are 3-deep
rings so DMA in, compute, and DMA out overlap across blocks.
"""

import sys

if "/opt/trn_rl_repo" not in sys.path:
    sys.path.insert(0, "/opt/trn_rl_repo")

import numpy as np

import concourse.bass as bass
import concourse.mybir as mybir
from concourse.bass_utils import run_bass_kernel_spmd

B, T, N = 32, 64, 16384
NCORES = 8
BL = B // NCORES  # local batches per core
SUB = 32  # neuron chunks per local batch
NL = N // SUB  # 512: free width of a timestep tile
P = BL * SUB  # 128 partitions
TBLK = 8  # timesteps per block
NBLK = T // TBLK
NRING = 3  # ring depth for x/spike/mem block tiles
F32 = mybir.dt.float32
ALU = mybir.AluOpType

_CACHE: dict = {}


def _build_nc() -> bass.Bass:
    nc = bass.Bass()
    x = nc.dram_tensor("x", [BL, T, N], F32, kind="ExternalInput")
    thresh = nc.dram_tensor("thresh", [N], F32, kind="ExternalInput")
    tau_x = nc.dram_tensor("tau_x", [N], F32, kind="ExternalInput")
    spikes = nc.dram_tensor("spikes", [BL, T, N], F32, kind="ExternalOutput")
    mems = nc.dram_tensor("mems", [BL, T, N], F32, kind="ExternalOutput")

    tau_2d = tau_x.rearrange("(s n) -> s n", n=NL)
    th_2d = thresh.rearrange("(s n) -> s n", n=NL)

    def x_src(b, k):
        # DRAM view of x[b, k*TBLK:(k+1)*TBLK, :] as [s, t, n]
        return x[b, k * TBLK : (k + 1) * TBLK, :].rearrange("t (s n) -> s t n", n=NL)

    def out_dst(dram, b, k):
        return dram[b, k * TBLK : (k + 1) * TBLK, :].rearrange(
            "t (s n) -> s t n", n=NL
        )

    def blk_view(tile, b):
        # [32, TBLK, NL] view of one local batch's partitions
        return tile[b * SUB : (b + 1) * SUB, :].rearrange("p (t n) -> p t n", n=NL)

    with (
        nc.sbuf_tensor([P, NL], F32) as tau_t,
        nc.sbuf_tensor([P, NL], F32) as sig_t,
        nc.sbuf_tensor([P, NL], F32) as th_t,
        nc.sbuf_tensor([P, NL], F32) as d_t,
        nc.sbuf_tensor([P, NL], F32) as e_t,
        nc.sbuf_tensor([P, NL], F32) as mp_t,
        nc.sbuf_tensor([P, NRING * TBLK * NL], F32) as xb_all,
        nc.sbuf_tensor([P, NRING * TBLK * NL], F32) as sb_all,
        nc.sbuf_tensor([P, NRING * TBLK * NL], F32) as mb_all,
        nc.semaphore("c_sem") as c_sem,
        nc.semaphore("act_sem") as act_sem,
        nc.semaphore("x_sem") as x_sem,
        nc.semaphore("xc_sem") as xc_sem,
        nc.semaphore("dve_sem") as dve_sem,
        nc.semaphore("so_sem") as so_sem,
        nc.semaphore("mo_sem") as mo_sem,
        nc.Block() as block,
    ):
        BW = TBLK * NL
        xb_t = [xb_all[:, r * BW : (r + 1) * BW] for r in range(NRING)]
        sb_t = [sb_all[:, r * BW : (r + 1) * BW] for r in range(NRING)]
        mb_t = [mb_all[:, r * BW : (r + 1) * BW] for r in range(NRING)]

        @block.sync
        def _(sync):
            # constants: [32, 512] source replicated to the 4 b-groups
            for b in range(BL):
                sync.dma_start(
                    out=tau_t[b * SUB : (b + 1) * SUB, :], in_=tau_2d
                ).then_inc(c_sem, 16)
            for b in range(BL):
                sync.dma_start(
                    out=th_t[b * SUB : (b + 1) * SUB, :], in_=th_2d
                ).then_inc(c_sem, 16)
            for k in range(NBLK):
                if k >= NRING:
                    # ring slot reuse: block k-NRING consumed by DVE
                    sync.wait_ge(xc_sem, k - NRING + 1)
                if k >= 1:
                    # issue throttle: previous block's DMAs fully done, so
                    # x_sem is at a full-count value before new increments.
                    sync.wait_ge(x_sem, 64 * k)
                for b in range(BL):
                    sync.dma_start(
                        out=blk_view(xb_t[k % NRING], b), in_=x_src(b, k)
                    ).then_inc(x_sem, 16)

        @block.scalar
        def _(scalar):
            scalar.wait_ge(c_sem, 16 * 2 * BL)
            nc.scalar.activation(
                sig_t[:], tau_t[:], mybir.ActivationFunctionType.Sigmoid
            ).then_inc(act_sem, 1)
            for k in range(NBLK):
                scalar.wait_ge(dve_sem, k + 1)
                if k >= 1:
                    scalar.wait_ge(so_sem, 64 * k)
                    scalar.wait_ge(mo_sem, 64 * k)
                for b in range(BL):
                    scalar.dma_start(
                        out=out_dst(spikes, b, k), in_=blk_view(sb_t[k % NRING], b)
                    ).then_inc(so_sem, 16)
                for b in range(BL):
                    scalar.dma_start(
                        out=out_dst(mems, b, k), in_=blk_view(mb_t[k % NRING], b)
                    ).then_inc(mo_sem, 16)
            scalar.wait_ge(so_sem, 64 * NBLK)
            scalar.wait_ge(mo_sem, 64 * NBLK)

        @block.vector
        def _(vector):
            vector.wait_ge(act_sem, 1)  # sig ready (implies thresh loaded)
            m_prev = None
            for k in range(NBLK):
                if k >= NRING:
                    # WAR: output DMAs of the block that used this ring slot
                    vector.wait_ge(so_sem, 64 * (k - NRING + 1))
                    vector.wait_ge(mo_sem, 64 * (k - NRING + 1))
                vector.wait_ge(x_sem, 64 * (k + 1))
                xb = xb_t[k % NRING]
                sb = sb_t[k % NRING]
                mb = mb_t[k % NRING]
                for tl in range(TBLK):
                    t = k * TBLK + tl
                    fsl = slice(tl * NL, (tl + 1) * NL)
                    xt = xb[:, fsl]
                    st = sb[:, fsl]
                    mt = mb[:, fsl]
                    last_of_block = tl == TBLK - 1
                    if t == 0:
                        # mem == 0: m' = x*sig
                        nc.vector.tensor_mul(out=mp_t[:], in0=xt, in1=sig_t[:])
                    else:
                        ins = nc.vector.tensor_sub(out=d_t[:], in0=xt, in1=m_prev)
                        if last_of_block:
                            ins.then_inc(xc_sem, 1)
                        nc.vector.tensor_mul(out=e_t[:], in0=d_t[:], in1=sig_t[:])
                        nc.vector.tensor_add(out=mp_t[:], in0=e_t[:], in1=m_prev)
                    nc.vector.tensor_tensor(
                        out=st, in0=mp_t[:], in1=th_t[:], op=ALU.is_ge
                    )
                    ins = nc.vector.scalar_tensor_tensor(
                        out=mt,
                        in0=st,
                        scalar=1.0,
                        in1=mp_t[:],
                        op0=ALU.is_lt,
                        op1=ALU.mult,
                    )
                    if last_of_block:
                        ins.then_inc(dve_sem, 1)
                        if k == 0:
                            # t==0 path above skipped the xc increment
                            pass
                    m_prev = mt
                if k == 0:
                    # block 0's xb consumption finished at its last sub; the
                    # t==0 special case means tl==7's sub carried the inc
                    # already (t=7 != 0), so nothing extra needed.
                    pass

    return nc


def _get_nc() -> bass.Bass:
    if "nc" not in _CACHE:
        _CACHE["nc"] = _build_nc()
    return _CACHE["nc"]


def kernel(x, thresh, tau_x, _trace: bool = False, _tmpdir: str | None = None):
    x = np.ascontiguousarray(np.asarray(x, dtype=np.float32))
    thresh = np.ascontiguousarray(np.asarray(thresh, dtype=np.float32))
    tau_x = np.ascontiguousarray(np.asarray(tau_x, dtype=np.float32))
    assert x.shape == (B, T, N)

    nc = _get_nc()
    in_maps = [
        {"x": x[i * BL : (i + 1) * BL], "thresh": thresh, "tau_x": tau_x}
        for i in range(NCORES)
    ]
    res = run_bass_kernel_spmd(
        nc, in_maps, core_ids=list(range(NCORES)), trace=_trace, tmpdir=_tmpdir
    )
    spikes = np.concatenate([r["spikes"] for r in res.results], axis=0)
    mems = np.concatenate([r["mems"] for r in res.results], axis=0)
    if _trace:
        _CACHE["last_results"] = res
    return spikes, mems

